# revision 1
# baseline (speedup 1.0000x reference)
"""Clustered attention Trainium2 kernel v2 — cluster-sparse (8-core SPMD).

Key ideas vs the dense baseline:
  * Host sorts positions by cluster label (per batch) and bin-packs clusters
    into groups of <=128 positions (descending size). Attention is
    block-diagonal in this order, so scores/exp/AV shrink ~4x.
  * The label mask is folded into the score matmul as a rank-3 accumulation:
    S' = sum_tot_g^T q + B*(u_c1 w_c1^T + u_c2 w_c2^T) - B, then
    exp(SCALE*S'). Same-cluster pairs get exp(SCALE*S); cross-cluster and
    slack rows get exp(<= -62) == 0 in bf16 — no DVE mask multiply at all.
  * Softmax denominator via a ones-column appended to V (the AV matmul
    yields [num | den] together); normalization happens on the host.
  * exp is batched over 2 (b,v) pairs ([128, 2, 512] PSUM) to halve the
    Activation-engine instruction count — Act is the per-pair critical
    engine once DMA is overlapped.
  * All HBM traffic in bf16 (~6.7 MB/core); big groups ride one merged
    slot-padded DMA per batch, small groups get exact row-sliced DMAs.
"""

import numpy as np
import ml_dtypes

import concourse.bass as bass
import concourse.tile as tile
from concourse import mybir
from concourse.bass_utils import run_bass_kernel_spmd

BF16 = ml_dtypes.bfloat16
F32 = np.float32

B, L, V, D = 2, 512, 64, 128
NCL = 8                      # number of cluster labels
N_CORES = 8
VC = V // N_CORES            # v slots per core
SCALE = 1.0 / float(np.sqrt(D))
BMASK = 1200.0               # mask bias; exp(SCALE*(s - BMASK)) == 0 in bf16
BIG = 96                     # groups >= BIG rows ride the merged padded DMA


_WAIT_EXEMPT = {
    "InstEventSemaphore", "InstNoOp", "InstCall", "InstISA",
    "InstUnconditionalBranch", "InstCompareAndBranch", "InstRegisterMove",
    "InstBranchHint", "InstHalt",
}


def _split_waits(nc, dma_cap=1, compute_cap=1):
    """walrus's sync-wait lowering tolerates 1 wait per instruction; hoist
    the excess onto preceding same-engine NoOps."""
    fn = nc.m.functions[0]
    for blk in fn.blocks:
        il = blk.instructions
        new = []
        changed = False
        for inst in il:
            tname = type(inst).__name__
            si = inst.sync_info
            if si is not None and tname not in _WAIT_EXEMPT:
                cap = dma_cap if tname in ("InstDMACopy", "InstDMA") else compute_cap
                waits = list(si.on_wait)
                if len(waits) > cap:
                    excess, keep = waits[:-cap], waits[-cap:]
                    for w in excess:
                        nop = mybir.InstNoOp(
                            name=nc.get_next_instruction_name(),
                            sync_info=mybir.SyncInfo(on_wait=[w], on_update=[]),
                            engine=inst.engine,
                            bass_nofuse=True,
                        )
                        new.append(nop)
                    inst.sync_info = mybir.SyncInfo(
                        on_wait=keep, on_update=list(si.on_update)
                    )
                    changed = True
            new.append(inst)
        if changed:
            blk.instructions = new


def _plan_from_labels(lab):
    """Per batch: sorted-by-cluster position order and group structure.

    plans[b] = (order, groups); groups is a list of (gglob, ioff, size,
    members) in DESCENDING size order. Greedy FFD bin-pack, cap 128, at most
    2 clusters per group (the mask matmul has 2 selector rows).
    """
    plans = []
    tot_g = 0
    for b in range(B):
        lb = np.asarray(lab[b]).astype(np.int64)
        counts = np.bincount(lb, minlength=NCL)
        order_all = np.argsort(lb, kind="stable")
        cstart = np.zeros(NCL + 1, np.int64)
        cstart[1:] = np.cumsum(counts)
        items = sorted(
            [(int(c), int(counts[c])) for c in range(NCL) if counts[c] > 0],
            key=lambda t: -t[1],
        )
        bins = []
        for c, s in items:
            assert s <= 128, "cluster larger than 128 unsupported"
            for bn in bins:
                if bn[0] + s <= 128 and len(bn[1]) < 2:
                    bn[0] += s
                    bn[1].append(c)
                    break
            else:
                bins.append([s, [c]])
        bins.sort(key=lambda bn: -bn[0])
        groups = []
        new_order = []
        ioff = 0
        for s, members in bins:
            pos = np.concatenate(
                [order_all[cstart[c]:cstart[c + 1]] for c in members]
            )
            groups.append((tot_g, ioff, s, tuple(members)))
            new_order.append(pos)
            ioff += s
            tot_g += 1
        plans.append((np.concatenate(new_order), groups))
    return plans, tot_g


def _build_bass(struct, cfg=None):
    """struct: per-b tuple of group sizes (descending). Program depends only
    on this (and cfg)."""
    if cfg is None:
        cfg = dict(
            batch=1, sp_bufs=3, av_bufs=5, pipe=2, ap_bufs=5, warm=False,
            order=["q01", "sA0", "WZ", "v0m", "v0s", "q28", "sA1",
                   "qb1a", "qb1b", "v1m", "v1s"],
            copy_assign={(1, "small"): "act"},
        )
    BATCH = cfg["batch"]
    nc = bass.Bass()
    bf = mybir.dt.bfloat16
    f32 = mybir.dt.float32

    tot_g = sum(len(s) for s in struct)
    g_of = []  # per b: list of (gglob, ioff, size, g_local)
    gg = 0
    for bsz in struct:
        lst = []
        io = 0
        for gl, s in enumerate(bsz):
            lst.append((gg, io, s, gl))
            io += s
            gg += 1
        assert io == L
        g_of.append(lst)
    nbig = [sum(1 for s in struct[b] if s >= BIG) for b in range(B)]

    sA = nc.dram_tensor("sA", (128, tot_g, 128), bf, kind="ExternalInput")
    WZ = nc.dram_tensor("WZ", (3, tot_g * 128 + B * L), bf, kind="ExternalInput")
    qm = nc.dram_tensor("qm", (128, B, VC, L), bf, kind="ExternalInput")
    # v/out dram in slot-padded layout: row index = g*128 + p
    vms = [
        nc.dram_tensor(f"v{b}", (len(struct[b]) * 128, VC, 129), bf,
                       kind="ExternalInput")
        for b in range(B)
    ]
    outs = [
        nc.dram_tensor("out0", (len(struct[0]) * 128, 129, VC), bf,
                       kind="ExternalOutput"),
        [nc.dram_tensor(f"out1{h}", (len(struct[1]) * 128, 129, VC // 2), bf,
                        kind="ExternalOutput") for h in ("a", "b")],
    ]

    with tile.TileContext(nc) as tc:
        with (
            tc.tile_pool(name="consts", bufs=1) as cpool,
            tc.tile_pool(name="ap", bufs=cfg.get("ap_bufs", 3)) as apool,
            tc.tile_pool(name="spsum", bufs=cfg["sp_bufs"], space="PSUM") as spsum,
            tc.tile_pool(name="avpsum", bufs=cfg["av_bufs"], space="PSUM") as avpool,
        ):
            s_sb = cpool.tile([128, tot_g, 128], bf)
            wz_sb = cpool.tile([3, tot_g * 128 + B * L], bf)
            q_sb = cpool.tile([128, B, VC, L], bf)
            v_sb = [
                cpool.tile([128, len(struct[b]), VC, 129], bf, name=f"v_sb{b}")
                for b in range(B)
            ]
            og0 = cpool.tile([128, len(struct[0]), 129, VC], bf)
            og1 = [
                cpool.tile([128, len(struct[1]), 129, VC // 2], bf,
                           name=f"og1{h}") for h in range(2)
            ]
            w_sb = wz_sb[:, 0:tot_g * 128].rearrange("p (g c) -> p g c", c=128)
            z_sb = wz_sb[:, tot_g * 128:].rearrange("p (b l) -> p b l", l=L)

            def v_dmas(b):
                nb = nbig[b]
                if nb:
                    nc.sync.dma_start(
                        out=v_sb[b][:, 0:nb, :, :],
                        in_=vms[b][0:nb * 128, :, :].rearrange(
                            "(g p) v c -> p g v c", p=128),
                    )
                for (gg_, io, s, gl) in g_of[b][nb:]:
                    nc.sync.dma_start(
                        out=v_sb[b][0:s, gl, :, :],
                        in_=vms[b][gl * 128:gl * 128 + s, :, :],
                    )

            # PE warmup: a 1-cycle matmul on const data pins pe_busy_start
            # at ~0.4us so the first real S batch runs at full clock.
            if cfg.get("warm", True):
                sp = spsum.tile([128, BATCH, L], f32)
                c0 = nc.const_aps.aps[(f32, 0.0)]
                nc.tensor.matmul(
                    sp[0:1, 0, 0:1], c0[:, 0:1], c0[:, 0:1],
                    start=True, stop=True,
                )

            # input DMAs (SP queue), ordered per cfg token list
            G0 = len(struct[0])

            def emit_in_dma(tok):
                if tok == "q01":
                    nc.sync.dma_start(out=q_sb[:, 0, 0:2, :], in_=qm[:, 0, 0:2, :])
                elif tok == "q24":
                    nc.sync.dma_start(out=q_sb[:, 0, 2:4, :], in_=qm[:, 0, 2:4, :])
                elif tok == "q48":
                    nc.sync.dma_start(out=q_sb[:, 0, 4:VC, :], in_=qm[:, 0, 4:VC, :])
                elif tok == "q46":
                    nc.sync.dma_start(out=q_sb[:, 0, 4:6, :], in_=qm[:, 0, 4:6, :])
                elif tok == "q68":
                    nc.sync.dma_start(out=q_sb[:, 0, 6:VC, :], in_=qm[:, 0, 6:VC, :])
                elif tok == "q28":
                    nc.sync.dma_start(out=q_sb[:, 0, 2:VC, :], in_=qm[:, 0, 2:VC, :])
                elif tok == "qb1":
                    nc.sync.dma_start(out=q_sb[:, 1, :, :], in_=qm[:, 1, :, :])
                elif tok == "qb1a":
                    nc.sync.dma_start(out=q_sb[:, 1, 0:4, :], in_=qm[:, 1, 0:4, :])
                elif tok == "qb1b":
                    nc.sync.dma_start(out=q_sb[:, 1, 4:VC, :], in_=qm[:, 1, 4:VC, :])
                elif tok == "sA0":
                    nc.sync.dma_start(out=s_sb[:, 0:G0, :], in_=sA[:, 0:G0, :])
                elif tok == "sA1":
                    nc.sync.dma_start(out=s_sb[:, G0:, :], in_=sA[:, G0:, :])
                elif tok == "WZ":
                    nc.sync.dma_start(out=wz_sb, in_=WZ[:, :])
                elif tok == "v0m":
                    nb = nbig[0]
                    nc.sync.dma_start(
                        out=v_sb[0][:, 0:nb, :, :],
                        in_=vms[0][0:nb * 128, :, :].rearrange(
                            "(g p) v c -> p g v c", p=128))
                elif tok == "v1m":
                    nb = nbig[1]
                    nc.sync.dma_start(
                        out=v_sb[1][:, 0:nb, :, :],
                        in_=vms[1][0:nb * 128, :, :].rearrange(
                            "(g p) v c -> p g v c", p=128))
                elif tok in ("v0s", "v1s"):
                    b = int(tok[1])
                    for (gg_, io, s, gl) in g_of[b][nbig[b]:]:
                        nc.sync.dma_start(
                            out=v_sb[b][0:s, gl, :, :],
                            in_=vms[b][gl * 128:gl * 128 + s, :, :])
                else:
                    raise ValueError(tok)

            for tok in cfg["order"]:
                emit_in_dma(tok)

            copy_state = [0]

            def av_tiles(b):
                nb = nbig[b]
                gs = g_of[b]
                tiles = []
                for i in range(0, nb, 3):
                    tiles.append(("big", i, gs[i:min(i + 3, nb)]))
                for i in range(nb, len(gs), 3):
                    tiles.append(("small", i, gs[i:i + 3]))
                return tiles

            def emit_av(job, cls):
                b, vh, at = job
                for pa in range(BATCH):
                    v = BATCH * vh + pa
                    for (tcls, slot0, tg) in av_tiles(b):
                        if tcls != cls:
                            continue
                        av = avpool.tile([128, len(tg), 129], f32)
                        for k, (gg_, io, s, gl) in enumerate(tg):
                            nc.tensor.matmul(
                                av[0:s, k, :], at[0:s, pa, io:io + s],
                                v_sb[b][0:s, gl, v, :],
                                start=True, stop=True,
                            )
                        copy_state[0] += 1
                        if b == 0:
                            dst = og0[:, slot0:slot0 + len(tg), :, v]
                        else:
                            dst = og1[v // (VC // 2)][
                                :, slot0:slot0 + len(tg), :, v % (VC // 2)]
                        # GPSIMD can't touch PSUM on hw: copies go to DVE,
                        # with a configurable share on Act (activation-Copy).
                        key = (b, cls)
                        mode = cfg.get("copy_assign", {}).get(key, "dve")
                        if v >= cfg.get("act_copy_vmax", VC):
                            mode = "dve"  # tail copies go to idle DVE
                        if mode == "act":
                            nc.scalar.copy(dst, av)
                        elif mode == "alt":
                            if copy_state[0] % 2:
                                nc.scalar.copy(dst, av)
                            else:
                                nc.vector.tensor_copy(dst, av)
                        else:
                            nc.vector.tensor_copy(dst, av)

            # Software-pipelined emission: PE sees S(batch k+pipe) before
            # AV(batch k), so it never head-of-line blocks on exp(k). Small
            # groups' v arrives later than the merged big-group DMA, so their
            # AVs ride a deeper pipe.
            from collections import deque
            pend_big = deque()
            pend_small = deque()
            PIPE = cfg.get("pipe", 1)
            PIPE_B1 = cfg.get("pipe_b1", PIPE)
            PIPE_S = cfg.get("pipe_small", PIPE)
            for b in range(B):
                groups = g_of[b]
                for vh in range(VC // BATCH):
                    sp = spsum.tile([128, BATCH, L], f32)
                    at = apool.tile([128, BATCH, L], bf)
                    for pa in range(BATCH):
                        v = BATCH * vh + pa
                        for (gg_, io, s, gl) in groups:
                            nc.tensor.matmul(
                                sp[:, pa, io:io + s], s_sb[:, gg_, :],
                                q_sb[:, b, v, io:io + s],
                                start=True, stop=False,
                            )
                            nc.tensor.matmul(
                                sp[:, pa, io:io + s], w_sb[:, gg_, :],
                                z_sb[:, b, io:io + s],
                                start=False, stop=True,
                            )
                    nc.scalar.activation(
                        at, sp, mybir.ActivationFunctionType.Exp, scale=SCALE,
                    )
                    pend_big.append((b, vh, at))
                    pend_small.append((b, vh, at))
                    pb = PIPE if pend_big[0][0] == 0 else PIPE_B1
                    if len(pend_big) > pb:
                        emit_av(pend_big.popleft(), "big")
                    if len(pend_small) > PIPE_S:
                        emit_av(pend_small.popleft(), "small")
            while pend_big:
                emit_av(pend_big.popleft(), "big")
            while pend_small:
                emit_av(pend_small.popleft(), "small")

            # out DMAs last (SP queue) so their sem waits never block compute
            out_ctr = [0]

            def out_dmas(dram, src_og, b):
                def q():
                    out_ctr[0] += 1
                    return nc.sync if out_ctr[0] % 2 else nc.scalar
                nb = nbig[b]
                if nb:
                    q().dma_start(
                        out=dram[0:nb * 128, :, :].rearrange(
                            "(g p) c v -> p g c v", p=128),
                        in_=src_og[:, 0:nb, :, :],
                    )
                for (gg_, io, s, gl) in g_of[b][nb:]:
                    q().dma_start(
                        out=dram[gl * 128:gl * 128 + s, :, :],
                        in_=src_og[0:s, gl, :, :],
                    )

            out_dmas(outs[0], og0, 0)
            out_dmas(outs[1][0], og1[0], 1)
            out_dmas(outs[1][1], og1[1], 1)
    _split_waits(nc)
    return nc


_BASS_CACHE = {}
_LAST_NC = None


def _get_bass(struct=None):
    global _LAST_NC
    if struct is None:
        if _LAST_NC is not None:
            return _LAST_NC
        struct = ((128, 128, 128, 128), (128, 128, 128, 128))
    if struct not in _BASS_CACHE:
        _BASS_CACHE[struct] = _build_bass(struct)
    _LAST_NC = _BASS_CACHE[struct]
    return _LAST_NC


def _prepare(query, key, value, label_arr):
    lab = np.asarray(label_arr)
    plans, tot_g = _plan_from_labels(lab)
    struct = tuple(tuple(s for (_, _, s, _) in p[1]) for p in plans)

    q = np.asarray(query, F32)
    val = np.asarray(value, F32)
    sumtot = np.asarray(key, F32).sum(axis=2)  # (B, L, D)

    sA = np.zeros((128, tot_g, 128), BF16)
    Wm = np.zeros((3, tot_g, 128), BF16)
    Zm = np.zeros((3, B, L), BF16)
    Wm[2, :, :] = BF16(-BMASK)
    Zm[2, :, :] = BF16(1.0)
    q_sorted = np.empty((B, L, V, D), F32)
    v_sorted = np.empty((B, L, V, D), F32)
    for b, (order, groups) in enumerate(plans):
        labs = lab[b][order]
        st = sumtot[b][order]          # (L, D)
        q_sorted[b] = q[b][order]
        v_sorted[b] = val[b][order]
        for (gg, io, s, members) in groups:
            sA[:, gg, :s] = st[io:io + s].T.astype(BF16)
            for k, c in enumerate(members):
                sel = (labs[io:io + s] == c)
                Wm[k, gg, :s] = (sel * BMASK).astype(BF16)
                Zm[k, b, io:io + s] = sel.astype(BF16)
    WZ = np.concatenate(
        [Wm.reshape(3, tot_g * 128), Zm.reshape(3, B * L)], axis=1
    )

    qb = q_sorted.astype(BF16)
    ones = np.ones((B, L, V, 1), BF16)
    vb = np.concatenate([v_sorted.astype(BF16), ones], axis=-1)  # (B,L,V,129)

    in_maps = []
    for c in range(N_CORES):
        sl = slice(c * VC, (c + 1) * VC)
        q_c = np.ascontiguousarray(qb[:, :, sl, :].transpose(3, 0, 2, 1))
        m = {"sA": sA, "WZ": WZ, "qm": q_c}
        for b in range(B):
            G = len(plans[b][1])
            vpad = np.zeros((G * 128, VC, 129), BF16)
            for (gg, io, s, gl) in (
                (gg, io, s, gl)
                for gl, (gg, io, s, _) in enumerate(plans[b][1])
            ):
                vpad[gl * 128:gl * 128 + s] = vb[b, io:io + s, sl, :]
            m[f"v{b}"] = vpad
        in_maps.append(m)
    return plans, struct, in_maps


def kernel(query, key, value, label_arr):
    plans, struct, in_maps = _prepare(query, key, value, label_arr)
    nc = _get_bass(struct)
    res = run_bass_kernel_spmd(nc, in_maps, core_ids=list(range(N_CORES)))
    full = np.empty((B, L, V, D), dtype=F32)
    for c in range(N_CORES):
        sl = slice(c * VC, (c + 1) * VC)
        for b in range(B):
            if b == 0:
                ob = np.asarray(res.results[c]["out0"], dtype=F32)
            else:
                ob = np.concatenate(
                    [np.asarray(res.results[c]["out1a"], dtype=F32),
                     np.asarray(res.results[c]["out1b"], dtype=F32)], axis=2)
            order, groups = plans[b]
            o_rows = np.empty((L, D, VC), F32)
            for gl, (gg, io, s, _) in enumerate(groups):
                blk = ob[gl * 128:gl * 128 + s]          # (s, 129, VC)
                o_rows[io:io + s] = blk[:, :D, :] / blk[:, D:D + 1, :]
            full[b, order, sl, :] = o_rows.transpose(0, 2, 1)
    return full



# revision 5
# speedup vs baseline: 1.0432x; 1.0432x over previous
"""Clustered attention Trainium2 kernel v3 — cluster-sparse (8-core SPMD).

Structure vs v2:
  * AV matmul is transposed: num[d, i] = sum_j v[j, d] * at[j, i] per group,
    so the AV PSUM tile is [128, L] (one bank, no 129-col group padding) and
    the output rides d-major (128, VC, L) dram tensors with zero slot padding.
  * Softmax denominator den[i] = sum_j at[j, i] comes from near-free N=1
    matmuls (lhsT = at, rhs = const ones column) into a shared PSUM tile,
    copied + DMA'd per batch (tiny).
  * exp is batched over 2 pairs ([128, 2, L] PSUM) to amortize Act overhead.
  * PSUM->SBUF copies are split between DVE and Act via cfg; out DMAs are
    per-2-pair chunks (2048B descriptors) issued as soon as their copies
    land, so the out stream chases the copy stream.
  * Small/aux DMAs can ride the Pool (SWDGE) queue to stay off the shared
    HWDGE pipeline stage.
"""

import numpy as np
import ml_dtypes

import concourse.bass as bass
import concourse.tile as tile
from concourse import mybir
from concourse.bass_utils import run_bass_kernel_spmd

BF16 = ml_dtypes.bfloat16
F32 = np.float32

B, L, V, D = 2, 512, 64, 128
NCL = 8                      # number of cluster labels
N_CORES = 8
VC = V // N_CORES            # v slots per core
SCALE = 1.0 / float(np.sqrt(D))
BMASK = 1200.0               # mask bias; exp(SCALE*(s - BMASK)) == 0 in bf16
BIG = 96                     # groups >= BIG rows ride the merged padded DMA


_WAIT_EXEMPT = {
    "InstEventSemaphore", "InstNoOp", "InstCall", "InstISA",
    "InstUnconditionalBranch", "InstCompareAndBranch", "InstRegisterMove",
    "InstBranchHint", "InstHalt",
}


def _split_waits(nc, dma_cap=1, compute_cap=1):
    """walrus's sync-wait lowering tolerates 1 wait per instruction; hoist
    the excess onto preceding same-engine NoOps."""
    fn = nc.m.functions[0]
    for blk in fn.blocks:
        il = blk.instructions
        new = []
        changed = False
        for inst in il:
            tname = type(inst).__name__
            si = inst.sync_info
            if si is not None and tname not in _WAIT_EXEMPT:
                cap = dma_cap if tname in ("InstDMACopy", "InstDMA") else compute_cap
                waits = list(si.on_wait)
                if len(waits) > cap:
                    excess, keep = waits[:-cap], waits[-cap:]
                    for w in excess:
                        nop = mybir.InstNoOp(
                            name=nc.get_next_instruction_name(),
                            sync_info=mybir.SyncInfo(on_wait=[w], on_update=[]),
                            engine=inst.engine,
                            bass_nofuse=True,
                        )
                        new.append(nop)
                    inst.sync_info = mybir.SyncInfo(
                        on_wait=keep, on_update=list(si.on_update)
                    )
                    changed = True
            new.append(inst)
        if changed:
            blk.instructions = new


def _plan_from_labels(lab):
    """Per batch: sorted-by-cluster position order and group structure.

    plans[b] = (order, groups); groups is a list of (gglob, ioff, size,
    members) in DESCENDING size order. Greedy FFD bin-pack, cap 128, at most
    2 clusters per group (the mask matmul has 2 selector rows).
    """
    plans = []
    tot_g = 0
    for b in range(B):
        lb = np.asarray(lab[b]).astype(np.int64)
        counts = np.bincount(lb, minlength=NCL)
        order_all = np.argsort(lb, kind="stable")
        cstart = np.zeros(NCL + 1, np.int64)
        cstart[1:] = np.cumsum(counts)
        items = sorted(
            [(int(c), int(counts[c])) for c in range(NCL) if counts[c] > 0],
            key=lambda t: -t[1],
        )
        bins = []
        for c, s in items:
            assert s <= 128, "cluster larger than 128 unsupported"
            for bn in bins:
                if bn[0] + s <= 128 and len(bn[1]) < 2:
                    bn[0] += s
                    bn[1].append(c)
                    break
            else:
                bins.append([s, [c]])
        bins.sort(key=lambda bn: -bn[0])
        groups = []
        new_order = []
        ioff = 0
        for s, members in bins:
            pos = np.concatenate(
                [order_all[cstart[c]:cstart[c + 1]] for c in members]
            )
            groups.append((tot_g, ioff, s, tuple(members)))
            new_order.append(pos)
            ioff += s
            tot_g += 1
        plans.append((np.concatenate(new_order), groups))
    return plans, tot_g


def _default_cfg():
    return dict(
        sp_bufs=2, av_bufs=3, ap_bufs=4, pipe=1, warm=True, cw=2,
        order=["q0:0:2", "sA", "WZ", "q0:2:8", "v0b", "v0s",
               "q1:0:4", "q1:4:8", "v1b", "v1s"],
        # queue per dma kind: sync (SP) / scalar (Act) / pool (GPSIMD)
        q_in="sync", q_vs="pool", q_out="sync", q_den="pool",
        # copy engine per (b, v): "dve" or "act"
        ceng={(0, 6): "act", (0, 7): "act",
              (1, 5): "act", (1, 6): "act", (1, 7): "act"},
    )


def _build_bass(struct, cfg=None):
    """struct: per-b tuple of group sizes (descending). Program depends only
    on this (and cfg)."""
    if cfg is None:
        cfg = _default_cfg()
    nc = bass.Bass()
    bf = mybir.dt.bfloat16
    f32 = mybir.dt.float32

    tot_g = sum(len(s) for s in struct)
    g_of = []  # per b: list of (gglob, ioff, size, g_local)
    gg = 0
    for bsz in struct:
        lst = []
        io = 0
        for gl, s in enumerate(bsz):
            lst.append((gg, io, s, gl))
            io += s
            gg += 1
        assert io == L
        g_of.append(lst)
    nbig = [sum(1 for s in struct[b] if s >= BIG) for b in range(B)]
    G = [len(struct[b]) for b in range(B)]
    # small-group dram row offsets (exact rows after the padded bigs)
    rows0 = []
    for b in range(B):
        r = nbig[b] * 128
        offs = {}
        for (gg_, io, s, gl) in g_of[b][nbig[b]:]:
            offs[gl] = r
            r += s
        rows0.append((offs, r))

    sA = nc.dram_tensor("sA", (128, tot_g, 128), bf, kind="ExternalInput")
    WZ = nc.dram_tensor("WZ", (3, tot_g * 128 + B * L), bf, kind="ExternalInput")
    qm = nc.dram_tensor("qm", (128, B, VC, L), bf, kind="ExternalInput")
    vms = [
        nc.dram_tensor(f"v{b}", (rows0[b][1], VC, 128), bf,
                       kind="ExternalInput")
        for b in range(B)
    ]
    outs = [
        nc.dram_tensor(f"out{b}", (128, VC, L), bf, kind="ExternalOutput")
        for b in range(B)
    ]
    dens = [
        nc.dram_tensor(f"den{b}", (128, VC, G[b]), bf, kind="ExternalOutput")
        for b in range(B)
    ]

    def q_of(kind):
        return {"sync": nc.sync, "scalar": nc.scalar,
                "pool": nc.gpsimd}[cfg[kind]]

    with tile.TileContext(nc) as tc:
        with (
            tc.tile_pool(name="consts", bufs=1) as cpool,
            tc.tile_pool(name="ap", bufs=cfg.get("ap_bufs", 4)) as apool,
            tc.tile_pool(name="spsum", bufs=cfg["sp_bufs"], space="PSUM") as spsum,
            tc.tile_pool(name="avpsum", bufs=cfg["av_bufs"], space="PSUM") as avpool,
            tc.tile_pool(name="denps", bufs=1, space="PSUM") as denpool,
        ):
            s_sb = cpool.tile([128, tot_g, 128], bf)
            wz_sb = cpool.tile([3, tot_g * 128 + B * L], bf)
            q_sb = cpool.tile([128, B, VC, L], bf)
            v_sb = [
                cpool.tile([128, G[b], VC, 128], bf, name=f"v_sb{b}")
                for b in range(B)
            ]
            og = cpool.tile([128, B, VC, L], bf)
            og_den = [
                cpool.tile([128, VC, G[b]], bf, name=f"og_den{b}")
                for b in range(B)
            ]
            w_sb = wz_sb[:, 0:tot_g * 128].rearrange("p (g c) -> p g c", c=128)
            z_sb = wz_sb[:, tot_g * 128:].rearrange("p (b l) -> p b l", l=L)
            den_ps = denpool.tile([128, B, VC, max(G)], f32)
            ones1 = nc.const_aps.aps[(bf, 1.0)]

            # PE warmup: a 1-cycle matmul on const data pins pe_busy_start
            # early so the first real S batch runs at full clock.
            if cfg.get("warm", True):
                spw = spsum.tile([128, 2, L], f32, name="sp")
                c0 = nc.const_aps.aps[(f32, 0.0)]
                nc.tensor.matmul(
                    spw[0:1, 0, 0:1], c0[:, 0:1], c0[:, 0:1],
                    start=True, stop=True,
                )

            def emit_in_dma(tok):
                if tok.startswith("q"):
                    b, v0, v1 = tok[1:].split(":")
                    b, v0, v1 = int(b), int(v0), int(v1)
                    q_of("q_in").dma_start(
                        out=q_sb[:, b, v0:v1, :], in_=qm[:, b, v0:v1, :])
                elif tok == "sA":
                    q_of("q_in").dma_start(out=s_sb, in_=sA[:, :, :])
                elif tok == "WZ":
                    q_of("q_in").dma_start(out=wz_sb, in_=WZ[:, :])
                elif tok in ("v0b", "v1b"):
                    b = int(tok[1])
                    nb = nbig[b]
                    q_of("q_in").dma_start(
                        out=v_sb[b][:, 0:nb, :, :],
                        in_=vms[b][0:nb * 128, :, :].rearrange(
                            "(g p) v d -> p g v d", p=128))
                elif tok in ("v0s", "v1s"):
                    b = int(tok[1])
                    for (gg_, io, s, gl) in g_of[b][nbig[b]:]:
                        r0 = rows0[b][0][gl]
                        q_of("q_vs").dma_start(
                            out=v_sb[b][0:s, gl, :, :],
                            in_=vms[b][r0:r0 + s, :, :])
                else:
                    raise ValueError(tok)

            for tok in cfg["order"]:
                emit_in_dma(tok)

            CW = cfg.get("cw", 2)

            def drain(job):
                b, vh, at = job
                for pa in range(2):
                    v = 2 * vh + pa
                    av = avpool.tile([128, L], f32)
                    for (gg_, io, s, gl) in g_of[b]:
                        nc.tensor.matmul(
                            av[:, io:io + s], v_sb[b][0:s, gl, v, :],
                            at[0:s, pa, io:io + s],
                            start=True, stop=True,
                        )
                        nc.tensor.matmul(
                            den_ps[0:s, b, v, gl:gl + 1], at[0:s, pa, io:io + s],
                            ones1[0:s, 0:1],
                            start=True, stop=True,
                        )
                    eng = cfg["ceng"].get((b, v), "dve")
                    dst = og[:, b, v, :]
                    if eng == "act":
                        nc.scalar.copy(dst, av)
                    else:
                        nc.vector.tensor_copy(dst, av)
                    if v % CW == CW - 1:
                        q_of("q_out").dma_start(
                            out=outs[b][:, v - CW + 1:v + 1, :],
                            in_=og[:, b, v - CW + 1:v + 1, :])
                if vh == VC // 2 - 1:  # batch b done: den copy + tiny DMA
                    nc.vector.tensor_copy(
                        og_den[b], den_ps[:, b, :, 0:G[b]])
                    q_of("q_den").dma_start(
                        out=dens[b][:, :, :], in_=og_den[b])

            from collections import deque
            pend = deque()
            PIPE = cfg.get("pipe", 1)
            for b in range(B):
                for vh in range(VC // 2):
                    sp = spsum.tile([128, 2, L], f32)
                    at = apool.tile([128, 2, L], bf)
                    for pa in range(2):
                        v = 2 * vh + pa
                        for (gg_, io, s, gl) in g_of[b]:
                            nc.tensor.matmul(
                                sp[:, pa, io:io + s], s_sb[:, gg_, :],
                                q_sb[:, b, v, io:io + s],
                                start=True, stop=False,
                            )
                            nc.tensor.matmul(
                                sp[:, pa, io:io + s], w_sb[:, gg_, :],
                                z_sb[:, b, io:io + s],
                                start=False, stop=True,
                            )
                    nc.scalar.activation(
                        at, sp, mybir.ActivationFunctionType.Exp, scale=SCALE,
                    )
                    pend.append((b, vh, at))
                    if len(pend) > PIPE:
                        drain(pend.popleft())
            while pend:
                drain(pend.popleft())
    _split_waits(nc)
    return nc


_BASS_CACHE = {}
_LAST_NC = None


def _get_bass(struct=None, cfg=None):
    global _LAST_NC
    if struct is None:
        if _LAST_NC is not None:
            return _LAST_NC
        struct = ((128, 128, 128, 128), (128, 128, 128, 128))
    key = (struct, repr(cfg))
    if key not in _BASS_CACHE:
        _BASS_CACHE[key] = _build_bass(struct, cfg)
    _LAST_NC = _BASS_CACHE[key]
    return _LAST_NC


def _prepare(query, key, value, label_arr):
    lab = np.asarray(label_arr)
    plans, tot_g = _plan_from_labels(lab)
    struct = tuple(tuple(s for (_, _, s, _) in p[1]) for p in plans)

    q = np.asarray(query, F32)
    val = np.asarray(value, F32)
    sumtot = np.asarray(key, F32).sum(axis=2)  # (B, L, D)

    sAm = np.zeros((128, tot_g, 128), BF16)
    Wm = np.zeros((3, tot_g, 128), BF16)
    Zm = np.zeros((3, B, L), BF16)
    Wm[2, :, :] = BF16(-BMASK)
    Zm[2, :, :] = BF16(1.0)
    q_sorted = np.empty((B, L, V, D), F32)
    v_sorted = np.empty((B, L, V, D), F32)
    nbig = []
    for b, (order, groups) in enumerate(plans):
        labs = lab[b][order]
        st = sumtot[b][order]          # (L, D)
        q_sorted[b] = q[b][order]
        v_sorted[b] = val[b][order]
        nbig.append(sum(1 for (_, _, s, _) in groups if s >= BIG))
        for (gg, io, s, members) in groups:
            sAm[:, gg, :s] = st[io:io + s].T.astype(BF16)
            for k, c in enumerate(members):
                sel = (labs[io:io + s] == c)
                Wm[k, gg, :s] = (sel * BMASK).astype(BF16)
                Zm[k, b, io:io + s] = sel.astype(BF16)
    WZ = np.concatenate(
        [Wm.reshape(3, tot_g * 128), Zm.reshape(3, B * L)], axis=1
    )

    qb = q_sorted.astype(BF16)
    vb = v_sorted.astype(BF16)       # (B, L, V, D)

    in_maps = []
    for c in range(N_CORES):
        sl = slice(c * VC, (c + 1) * VC)
        q_c = np.ascontiguousarray(qb[:, :, sl, :].transpose(3, 0, 2, 1))
        m = {"sA": sAm, "WZ": WZ, "qm": q_c}
        for b in range(B):
            groups = plans[b][1]
            nb = nbig[b]
            rows = nb * 128 + sum(s for (_, _, s, _) in groups[nb:])
            vpad = np.zeros((rows, VC, 128), BF16)
            r = nb * 128
            for gl, (gg, io, s, _) in enumerate(groups):
                if gl < nb:
                    vpad[gl * 128:gl * 128 + s] = vb[b, io:io + s, sl, :]
                else:
                    vpad[r:r + s] = vb[b, io:io + s, sl, :]
                    r += s
            m[f"v{b}"] = vpad
        in_maps.append(m)
    return plans, struct, in_maps


def kernel(query, key, value, label_arr):
    plans, struct, in_maps = _prepare(query, key, value, label_arr)
    nc = _get_bass(struct)
    res = run_bass_kernel_spmd(nc, in_maps, core_ids=list(range(N_CORES)))
    full = np.empty((B, L, V, D), dtype=F32)
    for c in range(N_CORES):
        sl = slice(c * VC, (c + 1) * VC)
        for b in range(B):
            num = np.asarray(res.results[c][f"out{b}"], dtype=F32)  # (128,VC,L)
            den = np.asarray(res.results[c][f"den{b}"], dtype=F32)  # (128,VC,G)
            order, groups = plans[b]
            o_rows = np.empty((L, VC, D), F32)
            for gl, (gg, io, s, _) in enumerate(groups):
                # num[d, v, io+p] / den[p, v, gl] -> (s, VC, D)
                blk = num[:, :, io:io + s].transpose(2, 1, 0)
                o_rows[io:io + s] = blk / den[:s, :, gl][:, :, None]
            full[b, order, sl, :] = o_rows
    return full


# revision 17
# speedup vs baseline: 1.1768x; 1.1281x over previous
"""Clustered attention Trainium2 kernel v3 — cluster-sparse (8-core SPMD).

Structure vs v2:
  * AV matmul is transposed: num[d, i] = sum_j v[j, d] * at[j, i] per group,
    so the AV PSUM tile is [128, L] (one bank, no 129-col group padding) and
    the output rides d-major (128, VC, L) dram tensors with zero slot padding.
  * Softmax denominator den[i] = sum_j at[j, i] comes from near-free N=1
    matmuls (lhsT = at, rhs = const ones column) into a shared PSUM tile,
    copied + DMA'd per batch (tiny).
  * exp is batched over 2 pairs ([128, 2, L] PSUM) to amortize Act overhead.
  * PSUM->SBUF copies are split between DVE and Act via cfg; out DMAs are
    per-2-pair chunks (2048B descriptors) issued as soon as their copies
    land, so the out stream chases the copy stream.
  * Small/aux DMAs can ride the Pool (SWDGE) queue to stay off the shared
    HWDGE pipeline stage.
"""

import numpy as np
import ml_dtypes

import concourse.bass as bass
import concourse.tile as tile
from concourse import mybir
from concourse.bass_utils import run_bass_kernel_spmd

BF16 = ml_dtypes.bfloat16
F32 = np.float32

B, L, V, D = 2, 512, 64, 128
NCL = 8                      # number of cluster labels
N_CORES = 8
VC = V // N_CORES            # v slots per core
SCALE = 1.0 / float(np.sqrt(D))
BMASK = 1200.0               # mask bias; exp(SCALE*(s - BMASK)) == 0 in bf16
BIG = 96                     # groups >= BIG rows ride the merged padded DMA


_WAIT_EXEMPT = {
    "InstEventSemaphore", "InstNoOp", "InstCall", "InstISA",
    "InstUnconditionalBranch", "InstCompareAndBranch", "InstRegisterMove",
    "InstBranchHint", "InstHalt",
}


def _split_waits(nc, dma_cap=1, compute_cap=1):
    """walrus's sync-wait lowering tolerates 1 wait per instruction; hoist
    the excess onto preceding same-engine NoOps."""
    fn = nc.m.functions[0]
    for blk in fn.blocks:
        il = blk.instructions
        new = []
        changed = False
        for inst in il:
            tname = type(inst).__name__
            si = inst.sync_info
            if si is not None and tname not in _WAIT_EXEMPT:
                cap = dma_cap if tname in ("InstDMACopy", "InstDMA") else compute_cap
                waits = list(si.on_wait)
                if len(waits) > cap:
                    excess, keep = waits[:-cap], waits[-cap:]
                    for w in excess:
                        nop = mybir.InstNoOp(
                            name=nc.get_next_instruction_name(),
                            sync_info=mybir.SyncInfo(on_wait=[w], on_update=[]),
                            engine=inst.engine,
                            bass_nofuse=True,
                        )
                        new.append(nop)
                    inst.sync_info = mybir.SyncInfo(
                        on_wait=keep, on_update=list(si.on_update)
                    )
                    changed = True
            new.append(inst)
        if changed:
            blk.instructions = new


def _plan_from_labels(lab):
    """Per batch: sorted-by-cluster position order and group structure.

    plans[b] = (order, groups); groups is a list of (gglob, ioff, size,
    members) in DESCENDING size order. Greedy FFD bin-pack, cap 128, at most
    2 clusters per group (the mask matmul has 2 selector rows).
    """
    plans = []
    tot_g = 0
    for b in range(B):
        lb = np.asarray(lab[b]).astype(np.int64)
        counts = np.bincount(lb, minlength=NCL)
        order_all = np.argsort(lb, kind="stable")
        cstart = np.zeros(NCL + 1, np.int64)
        cstart[1:] = np.cumsum(counts)
        items = sorted(
            [(int(c), int(counts[c])) for c in range(NCL) if counts[c] > 0],
            key=lambda t: -t[1],
        )
        bins = []
        for c, s in items:
            assert s <= 128, "cluster larger than 128 unsupported"
            for bn in bins:
                if bn[0] + s <= 128 and len(bn[1]) < 2:
                    bn[0] += s
                    bn[1].append(c)
                    break
            else:
                bins.append([s, [c]])
        bins.sort(key=lambda bn: -bn[0])
        groups = []
        new_order = []
        ioff = 0
        for s, members in bins:
            pos = np.concatenate(
                [order_all[cstart[c]:cstart[c + 1]] for c in members]
            )
            groups.append((tot_g, ioff, s, tuple(members)))
            new_order.append(pos)
            ioff += s
            tot_g += 1
        plans.append((np.concatenate(new_order), groups))
    return plans, tot_g


def _default_cfg():
    return dict(
        sp_bufs=2, av_bufs=3, ap_bufs=4, pipe=2, warm=True, cw=2,
        order=["sA", "q0:0:2", "WZ", "q0:2:4", "v0b", "q0:4:6", "q0:6:8",
               "v0s", "q1:0:2", "q1:2:4", "q1:4:6", "v1b", "q1:6:8", "v1s"],
        # queue per dma kind: sync (SP) / scalar (Act) / pool (GPSIMD)
        q_in="sync", q_vs="pool", q_out="sync", q_den="scalar",
        den_eng="act",
        # copy engine per (b, v): "dve" or "act"
        ceng={(1, 5): "act", (1, 6): "act", (1, 7): "act"},
    )


def _build_bass(struct, cfg=None):
    """struct: per-b tuple of group sizes (descending). Program depends only
    on this (and cfg)."""
    if cfg is None:
        cfg = _default_cfg()
    nc = bass.Bass()
    bf = mybir.dt.bfloat16
    f32 = mybir.dt.float32

    tot_g = sum(len(s) for s in struct)
    g_of = []  # per b: list of (gglob, ioff, size, g_local)
    gg = 0
    for bsz in struct:
        lst = []
        io = 0
        for gl, s in enumerate(bsz):
            lst.append((gg, io, s, gl))
            io += s
            gg += 1
        assert io == L
        g_of.append(lst)
    nbig = [sum(1 for s in struct[b] if s >= BIG) for b in range(B)]
    G = [len(struct[b]) for b in range(B)]
    # small-group dram row offsets (exact rows after the padded bigs)
    rows0 = []
    for b in range(B):
        r = nbig[b] * 128
        offs = {}
        for (gg_, io, s, gl) in g_of[b][nbig[b]:]:
            offs[gl] = r
            r += s
        rows0.append((offs, r))

    sA = nc.dram_tensor("sA", (128, tot_g, 128), bf, kind="ExternalInput")
    WZ = nc.dram_tensor("WZ", (3, tot_g * 128 + B * L), bf, kind="ExternalInput")
    qm = nc.dram_tensor("qm", (128, B, VC, L), bf, kind="ExternalInput")
    vms = [
        nc.dram_tensor(f"v{b}", (rows0[b][1], VC, 128), bf,
                       kind="ExternalInput")
        for b in range(B)
    ]
    outs = [
        nc.dram_tensor(f"out{b}", (128, VC, L), bf, kind="ExternalOutput")
        for b in range(B)
    ]
    dens = [
        nc.dram_tensor(f"den{b}", (128, VC, G[b]), bf, kind="ExternalOutput")
        for b in range(B)
    ]

    def q_of(kind):
        return {"sync": nc.sync, "scalar": nc.scalar,
                "pool": nc.gpsimd}[cfg[kind]]

    with tile.TileContext(nc) as tc:
        with (
            tc.tile_pool(name="consts", bufs=1) as cpool,
            tc.tile_pool(name="ap", bufs=cfg.get("ap_bufs", 4)) as apool,
            tc.tile_pool(name="spsum", bufs=cfg["sp_bufs"], space="PSUM") as spsum,
            tc.tile_pool(name="avpsum", bufs=cfg["av_bufs"], space="PSUM") as avpool,
            tc.tile_pool(name="denps", bufs=1, space="PSUM") as denpool,
        ):
            s_sb = cpool.tile([128, tot_g, 128], bf)
            wz_sb = cpool.tile([3, tot_g * 128 + B * L], bf)
            q_sb = cpool.tile([128, B, VC, L], bf)
            v_sb = [
                cpool.tile([128, G[b], VC, 128], bf, name=f"v_sb{b}")
                for b in range(B)
            ]
            og = cpool.tile([128, B, VC, L], bf)
            og_den = [
                cpool.tile([128, VC, G[b]], bf, name=f"og_den{b}")
                for b in range(B)
            ]
            w_sb = wz_sb[:, 0:tot_g * 128].rearrange("p (g c) -> p g c", c=128)
            z_sb = wz_sb[:, tot_g * 128:].rearrange("p (b l) -> p b l", l=L)
            den_ps = denpool.tile([128, B, VC, max(G)], f32)
            ones1 = nc.const_aps.aps[(bf, 1.0)]

            BATCH_B = cfg.get("batch_b", (cfg.get("batch", 2),) * B)
            BATCH = max(BATCH_B)

            # Dummy Pool work to delay SWDGE descriptor generation so the
            # early SP stream (sA/q) wins the DMA engines first.
            scr = cpool.tile([128, 4], bf)
            for _ in range(cfg.get("pool_delay", 0)):
                nc.gpsimd.memset(scr, 0.0)

            # PE warmup: chained matmuls on const data keep the PE busy until
            # the first real S matmul so it runs at a higher p-state.
            if cfg.get("warm", True):
                spw = spsum.tile([128, BATCH, L], f32, name="sp")
                c0 = nc.const_aps.aps[(f32, 0.0)]
                for wc in cfg.get("warm_chain", (1,)):
                    nc.tensor.matmul(
                        spw[0:1, 0, 0:wc], c0[:, 0:1],
                        c0[:, 0:1].to_broadcast((128, wc)) if wc > 1
                        else c0[:, 0:1],
                        start=True, stop=True,
                    )

            def emit_in_dma(tok):
                if tok.startswith("q"):
                    b, v0, v1 = tok[1:].split(":")
                    b, v0, v1 = int(b), int(v0), int(v1)
                    q_of("q_in").dma_start(
                        out=q_sb[:, b, v0:v1, :], in_=qm[:, b, v0:v1, :])
                elif tok == "sA":
                    q_of("q_in").dma_start(out=s_sb, in_=sA[:, :, :])
                elif tok == "WZ":
                    q_of("q_in").dma_start(out=wz_sb, in_=WZ[:, :])
                elif tok.startswith("v0b") or tok.startswith("v1b"):
                    b = int(tok[1])
                    nb = nbig[b]
                    g0, g1 = 0, nb
                    if ":" in tok:
                        _, a, z = tok.split(":")
                        g0, g1 = int(a), int(z)
                    q_of("q_in").dma_start(
                        out=v_sb[b][:, g0:g1, :, :],
                        in_=vms[b][g0 * 128:g1 * 128, :, :].rearrange(
                            "(g p) v d -> p g v d", p=128))
                elif tok in ("v0s", "v1s"):
                    b = int(tok[1])
                    for (gg_, io, s, gl) in g_of[b][nbig[b]:]:
                        r0 = rows0[b][0][gl]
                        q_of("q_vs").dma_start(
                            out=v_sb[b][0:s, gl, :, :],
                            in_=vms[b][r0:r0 + s, :, :])
                else:
                    raise ValueError(tok)

            for tok in cfg["order"]:
                emit_in_dma(tok)

            CW = cfg.get("cw", 2)

            def drain(job):
                b, vh, at = job
                for pa in range(BATCH_B[b]):
                    v = BATCH_B[b] * vh + pa
                    av = avpool.tile([128, L], f32)
                    for (gg_, io, s, gl) in g_of[b]:
                        nc.tensor.matmul(
                            av[:, io:io + s], v_sb[b][0:s, gl, v, :],
                            at[0:s, pa, io:io + s],
                            start=True, stop=True,
                        )
                        nc.tensor.matmul(
                            den_ps[0:s, b, v, gl:gl + 1], at[0:s, pa, io:io + s],
                            ones1[0:s, 0:1],
                            start=True, stop=True,
                        )
                    eng = cfg["ceng"].get((b, v), "dve")
                    dst = og[:, b, v, :]
                    if eng == "act":
                        nc.scalar.copy(dst, av)
                    else:
                        nc.vector.tensor_copy(dst, av)
                    if v == VC - 1:  # batch b done: den copy + tiny DMA
                        if cfg.get("den_eng", "dve") == "act":
                            nc.scalar.copy(og_den[b], den_ps[:, b, :, 0:G[b]])
                        else:
                            nc.vector.tensor_copy(
                                og_den[b], den_ps[:, b, :, 0:G[b]])
                        q_of("q_den").dma_start(
                            out=dens[b][:, :, :], in_=og_den[b])
                    if v % CW == CW - 1:
                        qo = cfg.get("q_out_map", {}).get((b, v), cfg["q_out"])
                        qe = {"sync": nc.sync, "scalar": nc.scalar,
                              "pool": nc.gpsimd}[qo]
                        qe.dma_start(
                            out=outs[b][:, v - CW + 1:v + 1, :],
                            in_=og[:, b, v - CW + 1:v + 1, :])

            from collections import deque
            pend = deque()
            PIPE_B = cfg.get("pipe_b", (cfg.get("pipe", 1),) * B)
            for b in range(B):
                NB = BATCH_B[b]
                for vh in range(VC // NB):
                    sp = spsum.tile([128, NB, L], f32, name="sp")
                    at = apool.tile([128, NB, L], bf, name="at")
                    for pa in range(NB):
                        v = NB * vh + pa
                        for (gg_, io, s, gl) in g_of[b]:
                            nc.tensor.matmul(
                                sp[:, pa, io:io + s], s_sb[:, gg_, :],
                                q_sb[:, b, v, io:io + s],
                                start=True, stop=False,
                            )
                            nc.tensor.matmul(
                                sp[:, pa, io:io + s], w_sb[:, gg_, :],
                                z_sb[:, b, io:io + s],
                                start=False, stop=True,
                            )
                    nc.scalar.activation(
                        at, sp, mybir.ActivationFunctionType.Exp, scale=SCALE,
                    )
                    pend.append((b, vh, at))
                    if len(pend) > PIPE_B[b]:
                        drain(pend.popleft())
            while pend:
                drain(pend.popleft())
    _split_waits(nc)
    return nc


_BASS_CACHE = {}
_LAST_NC = None


def _get_bass(struct=None, cfg=None):
    global _LAST_NC
    if struct is None:
        if _LAST_NC is not None:
            return _LAST_NC
        struct = ((128, 128, 128, 128), (128, 128, 128, 128))
    key = (struct, repr(cfg))
    if key not in _BASS_CACHE:
        _BASS_CACHE[key] = _build_bass(struct, cfg)
    _LAST_NC = _BASS_CACHE[key]
    return _LAST_NC


def _prepare(query, key, value, label_arr):
    lab = np.asarray(label_arr)
    plans, tot_g = _plan_from_labels(lab)
    struct = tuple(tuple(s for (_, _, s, _) in p[1]) for p in plans)

    q = np.asarray(query, F32)
    val = np.asarray(value, F32)
    sumtot = np.asarray(key, F32).sum(axis=2)  # (B, L, D)

    sAm = np.zeros((128, tot_g, 128), BF16)
    Wm = np.zeros((3, tot_g, 128), BF16)
    Zm = np.zeros((3, B, L), BF16)
    Wm[2, :, :] = BF16(-BMASK)
    Zm[2, :, :] = BF16(1.0)
    q_sorted = np.empty((B, L, V, D), F32)
    v_sorted = np.empty((B, L, V, D), F32)
    nbig = []
    for b, (order, groups) in enumerate(plans):
        labs = lab[b][order]
        st = sumtot[b][order]          # (L, D)
        q_sorted[b] = q[b][order]
        v_sorted[b] = val[b][order]
        nbig.append(sum(1 for (_, _, s, _) in groups if s >= BIG))
        for (gg, io, s, members) in groups:
            sAm[:, gg, :s] = st[io:io + s].T.astype(BF16)
            for k, c in enumerate(members):
                sel = (labs[io:io + s] == c)
                Wm[k, gg, :s] = (sel * BMASK).astype(BF16)
                Zm[k, b, io:io + s] = sel.astype(BF16)
    WZ = np.concatenate(
        [Wm.reshape(3, tot_g * 128), Zm.reshape(3, B * L)], axis=1
    )

    qb = q_sorted.astype(BF16)
    vb = v_sorted.astype(BF16)       # (B, L, V, D)

    in_maps = []
    for c in range(N_CORES):
        sl = slice(c * VC, (c + 1) * VC)
        q_c = np.ascontiguousarray(qb[:, :, sl, :].transpose(3, 0, 2, 1))
        m = {"sA": sAm, "WZ": WZ, "qm": q_c}
        for b in range(B):
            groups = plans[b][1]
            nb = nbig[b]
            rows = nb * 128 + sum(s for (_, _, s, _) in groups[nb:])
            vpad = np.zeros((rows, VC, 128), BF16)
            r = nb * 128
            for gl, (gg, io, s, _) in enumerate(groups):
                if gl < nb:
                    vpad[gl * 128:gl * 128 + s] = vb[b, io:io + s, sl, :]
                else:
                    vpad[r:r + s] = vb[b, io:io + s, sl, :]
                    r += s
            m[f"v{b}"] = vpad
        in_maps.append(m)
    return plans, struct, in_maps


def kernel(query, key, value, label_arr):
    plans, struct, in_maps = _prepare(query, key, value, label_arr)
    nc = _get_bass(struct)
    res = run_bass_kernel_spmd(nc, in_maps, core_ids=list(range(N_CORES)))
    full = np.empty((B, L, V, D), dtype=F32)
    for c in range(N_CORES):
        sl = slice(c * VC, (c + 1) * VC)
        for b in range(B):
            num = np.asarray(res.results[c][f"out{b}"], dtype=F32)  # (128,VC,L)
            den = np.asarray(res.results[c][f"den{b}"], dtype=F32)  # (128,VC,G)
            order, groups = plans[b]
            o_rows = np.empty((L, VC, D), F32)
            for gl, (gg, io, s, _) in enumerate(groups):
                # num[d, v, io+p] / den[p, v, gl] -> (s, VC, D)
                blk = num[:, :, io:io + s].transpose(2, 1, 0)
                o_rows[io:io + s] = blk / den[:s, :, gl][:, :, None]
            full[b, order, sl, :] = o_rows
    return full


# revision 38
# speedup vs baseline: 1.2143x; 1.0318x over previous
"""Clustered attention Trainium2 kernel v3 — cluster-sparse (8-core SPMD).

Structure vs v2:
  * AV matmul is transposed: num[d, i] = sum_j v[j, d] * at[j, i] per group,
    so the AV PSUM tile is [128, L] (one bank, no 129-col group padding) and
    the output rides d-major (128, VC, L) dram tensors with zero slot padding.
  * Softmax denominator den[i] = sum_j at[j, i] comes from near-free N=1
    matmuls (lhsT = at, rhs = const ones column) into a shared PSUM tile,
    copied + DMA'd per batch (tiny).
  * exp is batched over 2 pairs ([128, 2, L] PSUM) to amortize Act overhead.
  * PSUM->SBUF copies are split between DVE and Act via cfg; out DMAs are
    per-2-pair chunks (2048B descriptors) issued as soon as their copies
    land, so the out stream chases the copy stream.
  * Small/aux DMAs can ride the Pool (SWDGE) queue to stay off the shared
    HWDGE pipeline stage.
"""

import numpy as np
import ml_dtypes

import concourse.bass as bass
import concourse.tile as tile
from concourse import mybir
from concourse.bass_utils import run_bass_kernel_spmd

BF16 = ml_dtypes.bfloat16
F32 = np.float32

B, L, V, D = 2, 512, 64, 128
NCL = 8                      # number of cluster labels
N_CORES = 8
VC = V // N_CORES            # v slots per core
SCALE = 1.0 / float(np.sqrt(D))
BMASK = 1200.0               # mask bias; exp(SCALE*(s - BMASK)) == 0 in bf16
BIG = 96                     # groups >= BIG rows ride the merged padded DMA


_WAIT_EXEMPT = {
    "InstEventSemaphore", "InstNoOp", "InstCall", "InstISA",
    "InstUnconditionalBranch", "InstCompareAndBranch", "InstRegisterMove",
    "InstBranchHint", "InstHalt",
}


def _split_waits(nc, dma_cap=1, compute_cap=1):
    """walrus's sync-wait lowering tolerates 1 wait per instruction; hoist
    the excess onto preceding same-engine NoOps."""
    fn = nc.m.functions[0]
    for blk in fn.blocks:
        il = blk.instructions
        new = []
        changed = False
        for inst in il:
            tname = type(inst).__name__
            si = inst.sync_info
            if si is not None and tname not in _WAIT_EXEMPT:
                cap = dma_cap if tname in ("InstDMACopy", "InstDMA") else compute_cap
                waits = list(si.on_wait)
                if len(waits) > cap:
                    excess, keep = waits[:-cap], waits[-cap:]
                    for w in excess:
                        nop = mybir.InstNoOp(
                            name=nc.get_next_instruction_name(),
                            sync_info=mybir.SyncInfo(on_wait=[w], on_update=[]),
                            engine=inst.engine,
                            bass_nofuse=True,
                        )
                        new.append(nop)
                    inst.sync_info = mybir.SyncInfo(
                        on_wait=keep, on_update=list(si.on_update)
                    )
                    changed = True
            new.append(inst)
        if changed:
            blk.instructions = new


def _plan_from_labels(lab):
    """Per batch: sorted-by-cluster position order and group structure.

    plans[b] = (order, groups); groups is a list of (gglob, ioff, size,
    members) in DESCENDING size order. Greedy FFD bin-pack, cap 128, at most
    2 clusters per group (the mask matmul has 2 selector rows). Groups with
    a single cluster need no mask at all (the block is all same-label).
    """
    plans = []
    tot_g = 0
    for b in range(B):
        lb = np.asarray(lab[b]).astype(np.int64)
        counts = np.bincount(lb, minlength=NCL)
        order_all = np.argsort(lb, kind="stable")
        cstart = np.zeros(NCL + 1, np.int64)
        cstart[1:] = np.cumsum(counts)
        items = sorted(
            [(int(c), int(counts[c])) for c in range(NCL) if counts[c] > 0],
            key=lambda t: -t[1],
        )
        bins = []
        for c, s in items:
            assert s <= 128, "cluster larger than 128 unsupported"
            for bn in bins:
                if bn[0] + s <= 128 and len(bn[1]) < 2:
                    bn[0] += s
                    bn[1].append(c)
                    break
            else:
                bins.append([s, [c]])
        bins.sort(key=lambda bn: -bn[0])
        groups = []
        new_order = []
        ioff = 0
        for s, members in bins:
            pos = np.concatenate(
                [order_all[cstart[c]:cstart[c + 1]] for c in members]
            )
            groups.append((tot_g, ioff, s, tuple(members)))
            new_order.append(pos)
            ioff += s
            tot_g += 1
        plans.append((np.concatenate(new_order), groups))
    return plans, tot_g


def _default_cfg():
    return dict(
        sp_bufs=2, av_bufs=3, ap_bufs=4, pipe_b=(1, 2), warm=True, cw=2,
        order=["sA", "q0:0:2", "WZ", "q0:2:4", "v0b", "q0:4:6", "q0:6:8",
               "v0s", "q1:0:2", "v1b:0:1", "q1:2:4", "v1b:1:2", "q1:4:6",
               "v1b:2:3", "q1:6:8", "v1s"],
        # queue per dma kind: sync (SP) / scalar (Act) / pool (GPSIMD)
        q_in="sync", q_vs="pool", q_out="sync", q_den="scalar",
        den_eng="dve",
        # copy engine per (b, v): "dve" or "act"
        ceng={(1, 4): "act", (1, 6): "act"},
    )


def _build_bass(struct, cfg=None):
    """struct: per-b tuple of (group size, n clusters in group), descending
    size. Program depends only on this (and cfg)."""
    if cfg is None:
        cfg = _default_cfg()
    nc = bass.Bass()
    bf = mybir.dt.bfloat16
    f32 = mybir.dt.float32

    tot_g = sum(len(s) for s in struct)
    g_of = []  # per b: list of (gglob, ioff, size, g_local, nmem)
    gg = 0
    for bsz in struct:
        lst = []
        io = 0
        for gl, (s, nm) in enumerate(bsz):
            lst.append((gg, io, s, gl, nm))
            io += s
            gg += 1
        assert io == L
        g_of.append(lst)
    nbig = [sum(1 for (s, _) in struct[b] if s >= BIG) for b in range(B)]
    G = [len(struct[b]) for b in range(B)]
    # sA packed layout: paired groups keep 128 cols (zero-padded, M=128 with
    # the mask matmul closing the accumulation); single-cluster groups store
    # exactly s columns (M=s, lone start+stop matmul).
    sa_off = {}
    sa_cols = 0
    for b in range(B):
        for (gg_, io, s, gl, nm) in g_of[b]:
            sa_off[gg_] = sa_cols
            sa_cols += 128 if nm > 1 else s
    # small-group dram row offsets (exact rows after the padded bigs)
    rows0 = []
    for b in range(B):
        r = nbig[b] * 128
        offs = {}
        for (gg_, io, s, gl, nm) in g_of[b][nbig[b]:]:
            offs[gl] = r
            r += s
        rows0.append((offs, r))

    sA = nc.dram_tensor("sA", (128, sa_cols), bf, kind="ExternalInput")
    WZ = nc.dram_tensor("WZ", (3, tot_g * 128 + B * L), bf, kind="ExternalInput")
    qm = nc.dram_tensor("qm", (128, B, VC, L), bf, kind="ExternalInput")
    vms = [
        nc.dram_tensor(f"v{b}", (rows0[b][1], VC, 128), bf,
                       kind="ExternalInput")
        for b in range(B)
    ]
    outs = [
        nc.dram_tensor(f"out{b}", (128, VC, L), bf, kind="ExternalOutput")
        for b in range(B)
    ]
    dens = [
        nc.dram_tensor(f"den{b}", (128, VC, G[b]), bf, kind="ExternalOutput")
        for b in range(B)
    ]

    def q_of(kind):
        return {"sync": nc.sync, "scalar": nc.scalar,
                "pool": nc.gpsimd}[cfg[kind]]

    with tile.TileContext(nc) as tc:
        with (
            tc.tile_pool(name="consts", bufs=1) as cpool,
            tc.tile_pool(name="ap", bufs=cfg.get("ap_bufs", 4)) as apool,
            tc.tile_pool(name="spsum", bufs=cfg["sp_bufs"], space="PSUM") as spsum,
            tc.tile_pool(name="avpsum", bufs=cfg["av_bufs"], space="PSUM") as avpool,
            tc.tile_pool(name="denps", bufs=1, space="PSUM") as denpool,
        ):
            s_sb = cpool.tile([128, sa_cols], bf)
            wz_sb = cpool.tile([3, tot_g * 128 + B * L], bf)
            q_sb = cpool.tile([128, B, VC, L], bf)
            v_sb = [
                cpool.tile([128, G[b], VC, 128], bf, name=f"v_sb{b}")
                for b in range(B)
            ]
            og = cpool.tile([128, B, VC, L], bf)
            og_den = [
                cpool.tile([128, VC, G[b]], bf, name=f"og_den{b}")
                for b in range(B)
            ]
            w_sb = wz_sb[:, 0:tot_g * 128].rearrange("p (g c) -> p g c", c=128)
            z_sb = wz_sb[:, tot_g * 128:].rearrange("p (b l) -> p b l", l=L)
            den_ps = denpool.tile([128, B, VC, max(G)], f32)
            ones1 = nc.const_aps.aps[(bf, 1.0)]

            BATCH_B = cfg.get("batch_b", (cfg.get("batch", 2),) * B)
            BATCH = max(BATCH_B)

            # Dummy Pool work to delay SWDGE descriptor generation so the
            # early SP stream (sA/q) wins the DMA engines first.
            scr = cpool.tile([128, 4], bf)
            for _ in range(cfg.get("pool_delay", 0)):
                nc.gpsimd.memset(scr, 0.0)

            # PE warmup: chained matmuls on const data keep the PE busy until
            # the first real S matmul so it runs at a higher p-state.
            if cfg.get("warm", True):
                spw = spsum.tile([128, BATCH, L], f32, name="sp")
                c0 = nc.const_aps.aps[(f32, 0.0)]
                for wc in cfg.get("warm_chain", (1,)):
                    nc.tensor.matmul(
                        spw[0:1, 0, 0:wc], c0[:, 0:1],
                        c0[:, 0:1].to_broadcast((128, wc)) if wc > 1
                        else c0[:, 0:1],
                        start=True, stop=True,
                    )

            def emit_in_dma(tok):
                if tok.startswith("q"):
                    b, v0, v1 = tok[1:].split(":")
                    b, v0, v1 = int(b), int(v0), int(v1)
                    q_of("q_in").dma_start(
                        out=q_sb[:, b, v0:v1, :], in_=qm[:, b, v0:v1, :])
                elif tok == "sA":
                    q_of("q_in").dma_start(out=s_sb, in_=sA[:, :])
                elif tok == "WZ":
                    q_of("q_in").dma_start(out=wz_sb, in_=WZ[:, :])
                elif tok.startswith("v0b") or tok.startswith("v1b"):
                    b = int(tok[1])
                    nb = nbig[b]
                    g0, g1 = 0, nb
                    if ":" in tok:
                        _, a, z = tok.split(":")
                        g0, g1 = int(a), int(z)
                    q_of("q_in").dma_start(
                        out=v_sb[b][:, g0:g1, :, :],
                        in_=vms[b][g0 * 128:g1 * 128, :, :].rearrange(
                            "(g p) v d -> p g v d", p=128))
                elif tok in ("v0s", "v1s"):
                    b = int(tok[1])
                    for (gg_, io, s, gl, nm) in g_of[b][nbig[b]:]:
                        r0 = rows0[b][0][gl]
                        q_of("q_vs").dma_start(
                            out=v_sb[b][0:s, gl, :, :],
                            in_=vms[b][r0:r0 + s, :, :])
                else:
                    raise ValueError(tok)

            for tok in cfg["order"]:
                emit_in_dma(tok)

            CW = cfg.get("cw", 2)
            TILES_B = cfg.get(
                "tiles_b",
                tuple(tuple([BATCH_B[b]] * (VC // BATCH_B[b]))
                      for b in range(B)))
            CHUNKS_B = cfg.get(
                "chunks_b", tuple(tuple([CW] * (VC // CW)) for _ in range(B)))
            # map: (b, last v of chunk) -> (v0, v1)
            chunk_at = {}
            for b in range(B):
                v0c = 0
                for w in CHUNKS_B[b]:
                    chunk_at[(b, v0c + w - 1)] = (v0c, v0c + w)
                    v0c += w
                assert v0c == VC

            def drain(job):
                b, v0t, w, at = job
                for pa in range(w):
                    v = v0t + pa
                    av = avpool.tile([128, L], f32)
                    for (gg_, io, s, gl, nm) in g_of[b]:
                        nc.tensor.matmul(
                            av[:, io:io + s], v_sb[b][0:s, gl, v, :],
                            at[0:s, pa, io:io + s],
                            start=True, stop=True,
                        )
                        nc.tensor.matmul(
                            den_ps[0:s, b, v, gl:gl + 1], at[0:s, pa, io:io + s],
                            ones1[0:s, 0:1],
                            start=True, stop=True,
                        )
                    eng = cfg["ceng"].get((b, v), "dve")
                    dst = og[:, b, v, :]
                    if eng == "act":
                        nc.scalar.copy(dst, av)
                    else:
                        nc.vector.tensor_copy(dst, av)
                    if v == VC - 1:  # batch b done: den copy + tiny DMA
                        if cfg.get("den_eng", "dve") == "act":
                            nc.scalar.copy(og_den[b], den_ps[:, b, :, 0:G[b]])
                        else:
                            nc.vector.tensor_copy(
                                og_den[b], den_ps[:, b, :, 0:G[b]])
                        q_of("q_den").dma_start(
                            out=dens[b][:, :, :], in_=og_den[b])
                    if (b, v) in chunk_at:
                        c0, c1 = chunk_at[(b, v)]
                        qo = cfg.get("q_out_map", {}).get((b, v), cfg["q_out"])
                        qe = {"sync": nc.sync, "scalar": nc.scalar,
                              "pool": nc.gpsimd}[qo]
                        qe.dma_start(
                            out=outs[b][:, c0:c1, :],
                            in_=og[:, b, c0:c1, :])

            from collections import deque
            pend = deque()
            PIPE_B = cfg.get("pipe_b", (cfg.get("pipe", 1),) * B)
            for b in range(B):
                v0t = 0
                for w in TILES_B[b]:
                    sp = spsum.tile([128, w, L], f32, name="sp")
                    at = apool.tile([128, w, L], bf, name="at")
                    for pa in range(w):
                        v = v0t + pa
                        for (gg_, io, s, gl, nm) in g_of[b]:
                            # single-cluster groups: block is all same-label,
                            # no mask needed
                            c0s = sa_off[gg_]
                            mw = 128 if nm > 1 else s
                            nc.tensor.matmul(
                                sp[0:mw, pa, io:io + s], s_sb[:, c0s:c0s + mw],
                                q_sb[:, b, v, io:io + s],
                                start=True, stop=(nm == 1),
                            )
                            if nm > 1:
                                nc.tensor.matmul(
                                    sp[:, pa, io:io + s], w_sb[:, gg_, :],
                                    z_sb[:, b, io:io + s],
                                    start=False, stop=True,
                                )
                    nc.scalar.activation(
                        at, sp, mybir.ActivationFunctionType.Exp, scale=SCALE,
                    )
                    pend.append((b, v0t, w, at))
                    v0t += w
                    if len(pend) > PIPE_B[b]:
                        drain(pend.popleft())
            while pend:
                drain(pend.popleft())
    _split_waits(nc)
    return nc


_BASS_CACHE = {}
_LAST_NC = None
_LAST_CFG = None


def _get_bass(struct=None, cfg=None):
    global _LAST_NC, _LAST_CFG
    if struct is None:
        if _LAST_NC is not None:
            return _LAST_NC
        struct = (((128, 2),) * 4, ((128, 2),) * 4)
    key = (struct, repr(cfg))
    if key not in _BASS_CACHE:
        _BASS_CACHE[key] = _build_bass(struct, cfg)
    _LAST_NC = _BASS_CACHE[key]
    _LAST_CFG = cfg if cfg is not None else _default_cfg()
    return _LAST_NC


def _prepare(query, key, value, label_arr):
    lab = np.asarray(label_arr)
    plans, tot_g = _plan_from_labels(lab)
    struct = tuple(
        tuple((s, len(m)) for (_, _, s, m) in p[1]) for p in plans)

    q = np.asarray(query, F32)
    val = np.asarray(value, F32)
    sumtot = np.asarray(key, F32).sum(axis=2)  # (B, L, D)

    sa_cols = sum(
        (128 if len(m) > 1 else s) for p in plans for (_, _, s, m) in p[1])
    sAm = np.zeros((128, sa_cols), BF16)
    Wm = np.zeros((3, tot_g, 128), BF16)
    Zm = np.zeros((3, B, L), BF16)
    Wm[2, :, :] = BF16(-BMASK)
    Zm[2, :, :] = BF16(1.0)
    q_sorted = np.empty((B, L, V, D), F32)
    v_sorted = np.empty((B, L, V, D), F32)
    nbig = []
    sa_c = 0
    for b, (order, groups) in enumerate(plans):
        labs = lab[b][order]
        st = sumtot[b][order]          # (L, D)
        q_sorted[b] = q[b][order]
        v_sorted[b] = val[b][order]
        nbig.append(sum(1 for (_, _, s, _) in groups if s >= BIG))
        for (gg, io, s, members) in groups:
            sAm[:, sa_c:sa_c + s] = st[io:io + s].T.astype(BF16)
            sa_c += 128 if len(members) > 1 else s
            for k, c in enumerate(members):
                sel = (labs[io:io + s] == c)
                Wm[k, gg, :s] = (sel * BMASK).astype(BF16)
                Zm[k, b, io:io + s] = sel.astype(BF16)
    WZ = np.concatenate(
        [Wm.reshape(3, tot_g * 128), Zm.reshape(3, B * L)], axis=1
    )

    qb = q_sorted.astype(BF16)
    vb = v_sorted.astype(BF16)       # (B, L, V, D)

    in_maps = []
    for c in range(N_CORES):
        sl = slice(c * VC, (c + 1) * VC)
        q_c = np.ascontiguousarray(qb[:, :, sl, :].transpose(3, 0, 2, 1))
        m = {"sA": sAm, "WZ": WZ, "qm": q_c}
        for b in range(B):
            groups = plans[b][1]
            nb = nbig[b]
            rows = nb * 128 + sum(s for (_, _, s, _) in groups[nb:])
            vpad = np.zeros((rows, VC, 128), BF16)
            r = nb * 128
            for gl, (gg, io, s, _) in enumerate(groups):
                if gl < nb:
                    vpad[gl * 128:gl * 128 + s] = vb[b, io:io + s, sl, :]
                else:
                    vpad[r:r + s] = vb[b, io:io + s, sl, :]
                    r += s
            m[f"v{b}"] = vpad
        in_maps.append(m)
    return plans, struct, in_maps


def kernel(query, key, value, label_arr):
    plans, struct, in_maps = _prepare(query, key, value, label_arr)
    nc = _get_bass(struct)
    res = run_bass_kernel_spmd(nc, in_maps, core_ids=list(range(N_CORES)))
    full = np.empty((B, L, V, D), dtype=F32)
    for c in range(N_CORES):
        sl = slice(c * VC, (c + 1) * VC)
        for b in range(B):
            num = np.asarray(res.results[c][f"out{b}"], dtype=F32)  # (128,VC,L)
            den = np.asarray(res.results[c][f"den{b}"], dtype=F32)  # (128,VC,G)
            order, groups = plans[b]
            o_rows = np.empty((L, VC, D), F32)
            for gl, (gg, io, s, _) in enumerate(groups):
                # num[d, v, io+p] / den[p, v, gl] -> (s, VC, D)
                blk = num[:, :, io:io + s].transpose(2, 1, 0)
                o_rows[io:io + s] = blk / den[:s, :, gl][:, :, None]
            full[b, order, sl, :] = o_rows
    return full
